# revision 4
# baseline (speedup 1.0000x reference)
"""Trainium2 Bass kernel for nn_CrossAttention (no-softmax cross attention + residual + LayerNorm).

Reference computes:
    q = node @ W_q.T ; k = obs @ W_k.T ; v = obs @ W_v.T
    out = (q @ k.T) @ v ;  result = LayerNorm(out + node) * gamma + beta

Since there is no softmax, matmul associativity gives
    out + node = node @ (W_q.T @ W_k @ (obs.T @ obs) @ W_v.T + I) = node @ W_tot
which cuts 237 GFLOP to ~29 GFLOP (the headroom-8 reassociation).

Strategy (8 NeuronCores, SPMD):
  - Shard node rows 8 ways (6250 rows/core); replicate obs + weights.
  - Per core: small prelude builds W_tot = W_q.T @ W_k @ G @ W_v.T + I on-chip
    (G = obs.T @ obs Gram matrix contracts over obs rows = partition dim, so it
    needs no transpose; W_k/W_v are PE-transposed once).
  - Main loop streams 49 row-tiles of 128: DMA in -> PE transpose (node.T tiles)
    -> 4 accumulating fp32r matmuls against W_tot -> LayerNorm (bn_stats/bn_aggr
    on DVE, sqrt + normalize on ACT) -> DMA out.
  - fp32r (rounded-fp32 single-pass PE mode) measured at l2 rel-err ~1.5e-4 vs
    fp64, essentially identical to this HW's fp32 matmul, at 4x the speed.
"""

import numpy as np
from contextlib import ExitStack

import concourse.bacc as bacc
import concourse.bass as bass
import concourse.tile as tile
import concourse.mybir as mybir
import concourse.masks as masks

F32 = mybir.dt.float32
F32R = mybir.dt.float32r
AF = mybir.ActivationFunctionType
ALU = mybir.AluOpType

N_TOT, M, E, O = 50000, 2048, 512, 256
N_CORES = 8
NP = N_TOT // N_CORES          # 6250 rows per core
EPS = 1e-6
P = 128
KE = E // P                    # 4 contraction tiles over E
KO = O // P                    # 2 contraction tiles over O
MT = M // P                    # 16 obs row tiles
NT = (NP + P - 1) // P         # 49 node row tiles per core
LAST = NP - (NT - 1) * P       # 106 rows in the last tile


def _build(apply_affine: bool):
    nc = bacc.Bacc("TRN2", target_bir_lowering=False, debug=False,
                   num_devices=N_CORES)
    node = nc.dram_tensor("node", [NP, E], F32, kind="ExternalInput")
    obs = nc.dram_tensor("obs", [M, O], F32, kind="ExternalInput")
    wq = nc.dram_tensor("wq", [E, E], F32, kind="ExternalInput")
    wk = nc.dram_tensor("wk", [E, O], F32, kind="ExternalInput")
    wv = nc.dram_tensor("wv", [E, O], F32, kind="ExternalInput")
    if apply_affine:
        gam = nc.dram_tensor("gam", [1, E], F32, kind="ExternalInput")
        bet = nc.dram_tensor("bet", [1, E], F32, kind="ExternalInput")
    out = nc.dram_tensor("out", [NP, E], F32, kind="ExternalOutput")

    with tile.TileContext(nc) as tc, ExitStack() as ctx:
        const = ctx.enter_context(tc.tile_pool(name="const", bufs=1))
        wtot_pool = ctx.enter_context(tc.tile_pool(name="wtotp", bufs=1))

        ident = const.tile([P, P], F32)
        masks.make_identity(nc, ident[:])
        eps_t = const.tile([P, 1], F32)
        nc.gpsimd.memset(eps_t[:], EPS)

        wtot = wtot_pool.tile([P, KE, E], F32R)   # W_tot, k-tiled over rows
        if apply_affine:
            gbc = const.tile([P, E], F32)         # gamma broadcast
            bbc = const.tile([P, E], F32)         # beta broadcast

        # ---------------- prelude: W_tot = W_q.T @ W_k @ G @ W_v.T + I -------
        with ExitStack() as pctx:
            sc = pctx.enter_context(tc.tile_pool(name="presb", bufs=1))
            pps = pctx.enter_context(
                tc.tile_pool(name="preps", bufs=4, space="PSUM"))

            # identity in f32r + shifted identity block for the +I fold
            ident_r = sc.tile([P, P], F32R)
            nc.vector.tensor_copy(ident_r[:], ident[:])
            zsh = sc.tile([P, 2 * KE * P], F32)    # [128, 1024], I at cols [512:640)
            nc.gpsimd.memset(zsh[:], 0.0)
            nc.gpsimd.affine_select(
                out=zsh[:, KE * P:(KE + 1) * P], in_=zsh[:, KE * P:(KE + 1) * P],
                compare_op=ALU.not_equal, fill=1.0, base=0,
                pattern=[[-1, P]], channel_multiplier=1)
            zsh_r = sc.tile([P, 2 * KE * P], F32R)
            nc.vector.tensor_copy(zsh_r[:], zsh[:])

            obs_sb = sc.tile([P, MT, O], F32R)
            nc.sync.dma_start(
                obs_sb[:], obs.ap().rearrange("(t p) o -> p t o", p=P).bitcast(F32R))
            wq_sb = sc.tile([P, KE, E], F32R)
            nc.sync.dma_start(
                wq_sb[:], wq.ap().rearrange("(k p) x -> p k x", p=P).bitcast(F32R))
            wk_sb = sc.tile([P, KE, O], F32)
            nc.sync.dma_start(
                wk_sb[:], wk.ap().rearrange("(k p) o -> p k o", p=P))
            wv_sb = sc.tile([P, KE, O], F32)
            nc.sync.dma_start(
                wv_sb[:], wv.ap().rearrange("(k p) o -> p k o", p=P))

            # G = obs.T @ obs   [256, 256]
            g_sb = sc.tile([P, KO, O], F32R)
            for a in range(KO):
                g_ps = pps.tile([P, O], F32, tag="pps")
                for t in range(MT):
                    nc.tensor.matmul(
                        g_ps[:], obs_sb[:, t, a * P:(a + 1) * P], obs_sb[:, t, :],
                        start=(t == 0), stop=(t == MT - 1))
                nc.scalar.copy(g_sb[:, a, :], g_ps[:])

            # W_v.T and W_k.T  [256, 512] each, via PE transpose
            wvT_sb = sc.tile([P, KO, E], F32R)
            wkT_sb = sc.tile([P, KO, E], F32R)
            for (src, dst) in ((wv_sb, wvT_sb), (wk_sb, wkT_sb)):
                for b in range(KO):
                    t_ps = pps.tile([P, E], F32, tag="pps")
                    for j in range(KE):
                        nc.tensor.transpose(
                            t_ps[:, j * P:(j + 1) * P],
                            src[:, j, b * P:(b + 1) * P], ident[:])
                    nc.scalar.copy(dst[:, b, :], t_ps[:])

            # T1 = G @ W_v.T  [256, 512]  (G symmetric -> G tiles usable as lhsT)
            t1_sb = sc.tile([P, KO, E], F32R)
            for a in range(KO):
                t1_ps = pps.tile([P, E], F32, tag="pps")
                for b in range(KO):
                    nc.tensor.matmul(
                        t1_ps[:], g_sb[:, b, a * P:(a + 1) * P], wvT_sb[:, b, :],
                        start=(b == 0), stop=(b == KO - 1))
                nc.scalar.copy(t1_sb[:, a, :], t1_ps[:])

            # T2 = W_k @ T1  [512, 512]
            t2_sb = sc.tile([P, KE, E], F32R)
            for x in range(KE):
                t2_ps = pps.tile([P, E], F32, tag="pps")
                for b in range(KO):
                    nc.tensor.matmul(
                        t2_ps[:], wkT_sb[:, b, x * P:(x + 1) * P], t1_sb[:, b, :],
                        start=(b == 0), stop=(b == KO - 1))
                nc.scalar.copy(t2_sb[:, x, :], t2_ps[:])

            # W_tot = W_q.T @ T2 + I  [512, 512]
            for x in range(KE):
                w_ps = pps.tile([P, E], F32, tag="pps")
                for k in range(KE):
                    nc.tensor.matmul(
                        w_ps[:], wq_sb[:, k, x * P:(x + 1) * P], t2_sb[:, k, :],
                        start=(k == 0), stop=False)
                nc.tensor.matmul(
                    w_ps[:], ident_r[:],
                    zsh_r[:, KE * P - x * P: 2 * KE * P - x * P],
                    start=False, stop=True)
                nc.scalar.copy(wtot[:, x, :], w_ps[:])

            if apply_affine:
                ones_r = sc.tile([1, P], F32R)
                nc.gpsimd.memset(ones_r[:], 1.0)
                gam_sb = sc.tile([1, E], F32R)
                nc.sync.dma_start(gam_sb[:], gam.ap().bitcast(F32R))
                bet_sb = sc.tile([1, E], F32R)
                nc.sync.dma_start(bet_sb[:], bet.ap().bitcast(F32R))
                for (src, dst) in ((gam_sb, gbc), (bet_sb, bbc)):
                    bc_ps = pps.tile([P, E], F32, tag="pps")
                    nc.tensor.matmul(bc_ps[:], ones_r[:], src[:])
                    nc.scalar.copy(dst[:], bc_ps[:])

        # ---------------- main loop over node row tiles ----------------------
        node_pool = ctx.enter_context(tc.tile_pool(name="nodep", bufs=4))
        ndT_pool = ctx.enter_context(tc.tile_pool(name="ndtp", bufs=3))
        out_pool = ctx.enter_context(tc.tile_pool(name="outp", bufs=4))
        stat_pool = ctx.enter_context(tc.tile_pool(name="statp", bufs=4))
        psT_pool = ctx.enter_context(
            tc.tile_pool(name="pstp", bufs=2, space="PSUM"))
        acc_pool = ctx.enter_context(
            tc.tile_pool(name="accp", bufs=2, space="PSUM"))

        node_ap = node.ap()
        out_ap = out.ap()
        for t in range(NT):
            r0 = t * P
            rn = P if t < NT - 1 else LAST

            nd = node_pool.tile([P, E], F32)
            nc.sync.dma_start(nd[:rn], node_ap[r0:r0 + rn, :])

            psT = psT_pool.tile([P, E], F32)
            for k in range(KE):
                nc.tensor.transpose(
                    psT[:, k * P:k * P + rn], nd[:rn, k * P:(k + 1) * P],
                    ident[:rn, :rn])
            ndT = ndT_pool.tile([P, E], F32R)
            if rn == P:
                nc.scalar.copy(ndT[:], psT[:])
            else:
                for k in range(KE):
                    nc.scalar.copy(ndT[:, k * P:k * P + rn],
                                   psT[:, k * P:k * P + rn])

            acc = acc_pool.tile([P, E], F32)
            for k in range(KE):
                nc.tensor.matmul(
                    acc[:rn, :], ndT[:, k * P:k * P + rn], wtot[:, k, :],
                    start=(k == 0), stop=(k == KE - 1))

            # LayerNorm over the free dim
            bn6 = stat_pool.tile([P, 6], F32)
            nc.vector.bn_stats(bn6[:rn], acc[:rn, :])
            mv = stat_pool.tile([P, 2], F32)
            nc.vector.bn_aggr(mv[:rn], bn6[:rn])
            std = stat_pool.tile([P, 1], F32)
            nc.scalar.activation(std[:rn], mv[:rn, 1:2], AF.Sqrt,
                                 bias=eps_t[:rn], scale=1.0)
            rstd = stat_pool.tile([P, 1], F32)
            nc.vector.reciprocal(rstd[:rn], std[:rn])
            nmr = stat_pool.tile([P, 1], F32)   # -mean * rstd
            nc.vector.tensor_scalar(nmr[:rn], mv[:rn, 0:1], rstd[:rn], -1.0,
                                    ALU.mult, ALU.mult)
            ot = out_pool.tile([P, E], F32)
            nc.scalar.activation(ot[:rn], acc[:rn, :], AF.Identity,
                                 bias=nmr[:rn], scale=rstd[:rn])
            if apply_affine:
                nc.vector.tensor_mul(ot[:rn], ot[:rn], gbc[:rn])
                nc.vector.tensor_add(ot[:rn], ot[:rn], bbc[:rn])

            nc.sync.dma_start(out_ap[r0:r0 + rn, :], ot[:rn])

    nc.compile()
    return nc


_CACHE: dict = {}


def _get_runner(apply_affine: bool):
    """Build + jit once; returns a callable(list_of_in_maps) -> np [N_TOT, E]."""
    key = apply_affine
    if key in _CACHE:
        return _CACHE[key]

    import jax
    from jax.sharding import Mesh, PartitionSpec
    from jax.experimental.shard_map import shard_map
    from concourse import bass2jax

    nc = _build(apply_affine)
    bass2jax.install_neuronx_cc_hook()

    partition_name = (nc.partition_id_tensor.name
                      if nc.partition_id_tensor else None)
    in_names, out_names, out_avals, zero_outs = [], [], [], []
    for alloc in nc.m.functions[0].allocations:
        if not isinstance(alloc, mybir.MemoryLocationSet):
            continue
        name = alloc.memorylocations[0].name
        if alloc.kind == "ExternalInput":
            if name != partition_name:
                in_names.append(name)
        elif alloc.kind == "ExternalOutput":
            shape = tuple(alloc.tensor_shape)
            dtype = mybir.dt.np(alloc.dtype)
            out_names.append(name)
            out_avals.append(jax.core.ShapedArray(shape, dtype))
            zero_outs.append(np.zeros(shape, dtype))
    n_params = len(in_names)
    all_names = in_names + out_names
    if partition_name is not None:
        all_names = all_names + [partition_name]
    donate = tuple(range(n_params, n_params + len(out_names)))

    def _body(*args):
        operands = list(args)
        if partition_name is not None:
            operands.append(bass2jax.partition_id_tensor())
        outs = bass2jax._bass_exec_p.bind(
            *operands,
            out_avals=tuple(out_avals),
            in_names=tuple(all_names),
            out_names=tuple(out_names),
            lowering_input_output_aliases=(),
            sim_require_finite=True,
            sim_require_nnan=True,
            nc=nc,
        )
        return tuple(outs)

    devices = jax.devices()[:N_CORES]
    mesh = Mesh(np.asarray(devices), ("core",))
    n_io = n_params + len(out_names)
    mapped = shard_map(_body, mesh=mesh,
                       in_specs=(PartitionSpec("core"),) * n_io,
                       out_specs=(PartitionSpec("core"),) * len(out_names))
    sharded = jax.jit(mapped, donate_argnums=donate, keep_unused=True)
    sharded_t = jax.jit(mapped, keep_unused=True)  # non-donating, reusable args

    shardings = [jax.sharding.NamedSharding(mesh, PartitionSpec("core"))] * n_io

    def put(arrs):
        return [jax.device_put(a, s) for a, s in zip(arrs, shardings)]

    runner = {
        "fn": sharded,
        "fn_t": sharded_t,
        "put": put,
        "in_names": in_names,
        "out_names": out_names,
        "zero_outs": zero_outs,
    }
    _CACHE[key] = runner
    return runner


def _prep_inputs(runner, inputs_np: dict) -> list:
    """Concat per-core inputs along axis 0 (global arrays for shard_map)."""
    concat = []
    for name in runner["in_names"]:
        per_core = inputs_np[name]           # list of 8 per-core arrays
        concat.append(np.concatenate(per_core, axis=0))
    for z in runner["zero_outs"]:
        concat.append(np.zeros((N_CORES * z.shape[0], *z.shape[1:]), z.dtype))
    return concat


def _make_per_core(node_feature, obs_feature, W_q, W_k, W_v, gam, bet,
                   apply_affine):
    f = np.ascontiguousarray
    per = {
        "node": [f(node_feature[c * NP:(c + 1) * NP]) for c in range(N_CORES)],
        "obs": [f(obs_feature)] * N_CORES,
        "wq": [f(W_q)] * N_CORES,
        "wk": [f(W_k)] * N_CORES,
        "wv": [f(W_v)] * N_CORES,
    }
    if apply_affine:
        per["gam"] = [f(gam.reshape(1, E))] * N_CORES
        per["bet"] = [f(bet.reshape(1, E))] * N_CORES
    return per


def kernel(node_feature, obs_feature, W_q, W_k, W_v, ln_gamma, ln_beta):
    node_feature = np.asarray(node_feature, dtype=np.float32)
    obs_feature = np.asarray(obs_feature, dtype=np.float32)
    W_q = np.asarray(W_q, dtype=np.float32)
    W_k = np.asarray(W_k, dtype=np.float32)
    W_v = np.asarray(W_v, dtype=np.float32)
    ln_gamma = np.asarray(ln_gamma, dtype=np.float32)
    ln_beta = np.asarray(ln_beta, dtype=np.float32)

    apply_affine = not (np.all(ln_gamma == 1.0) and np.all(ln_beta == 0.0))
    runner = _get_runner(apply_affine)
    per = _make_per_core(node_feature, obs_feature, W_q, W_k, W_v,
                         ln_gamma, ln_beta, apply_affine)
    args = _prep_inputs(runner, per)
    outs = runner["fn"](*args)
    res = np.asarray(outs[runner["out_names"].index("out")])
    return res.reshape(N_TOT, E)
